# revision 21
# baseline (speedup 1.0000x reference)
"""Trainium2 Bass kernel for multi-head enc-dec attention with softmax over
the query axis (legacy F.softmax(dim=1) on [N, S, S]) plus an output
projection.

Math (per head n):
    S[i, j]  = sum_d Q[n, d, i] * K[n, d, j] / sqrt(128)
    E        = exp(S)                      (softmax over i == axis 0)
    U[d, j]  = sum_i V[n, d, i] * E[i, j]
    out_h    = U / colsum(E)               (colsum over i, per j)
    result[b] = sum_{heads h of b} W_h^T @ out_h

Sharding: N = 64 heads split across 8 cores (8 heads each). No collectives.

v4 design (device = the 99.2% of FLOPs that are S-sized; host = the
final per-j normalization + d_model projection, 0.8% of FLOPs):
  - The scalar engine (exp of all 4.2M scores per head) is the hard
    bottleneck at ~456ns/512 cols + 316ns/instruction (measured), so the
    schedule exists to keep ScalarE saturated: scores are computed in
    [128, 512] pieces packed 3-per-[128,1536]-PSUM-tile so each Exp
    instruction covers 1536 columns.
  - mm2 (V^T E) accumulates per-head U in PSUM; adjacent (js0, js1)
    pieces of one i-chunk are paired into single 1024-wide matmuls.
  - colsum of E via 8 tiny ones-matmuls (lhsT = esum 128-slices) into a
    [128, 8] PSUM tile => per-j sums land transposed; DMA'd out raw.
  - U and colsumT stream to DRAM; the host does r = 1/colsum,
    out_b += W_h^T (U_h * r) in fp32 (also slightly more accurate than
    a device-side bf16 projection).
PSUM budget: 2 x 3 banks scores + 2 banks U = 8; the colsumT tile
briefly borrows a score-ring slot (lifetime < 2 exp ops).
"""

import math
from contextlib import ExitStack

import ml_dtypes
import numpy as np

N_CORES = 8
B, N_HEADS, D, S = 4, 16, 128, 2048
HPC = (B * N_HEADS) // N_CORES  # heads per core = 8
IC = S // 128  # 16 i-chunks
JH = 2  # j halves
JHALF = S // JH  # 1024
NSTEP = HPC * JH
NPIECE = 2 * IC  # 512-wide (ic, js) score pieces per step
FUSE = 3  # pieces per PSUM score tile / Exp op
NBIG = (NPIECE + FUSE - 1) // FUSE
SCALE = 1.0 / math.sqrt(D)
LAG = 12  # mm2 piece lag
RUNWAY = 3  # bigs at step start during which mm2 drain is paused
TAIL_SCHED = [1, 2, 2, 3, 4]  # big (in next step) per tail piece
QK_BF16 = True

_COMPILED = {}


def _build_nc(loop_n=None):
    """loop_n: if set, wrap the body in a device-side For_i that repeats it
    loop_n times (used only for wall-clock-difference HW timing)."""
    import contextlib
    import concourse.mybir as mybir
    import concourse.tile as tile
    from concourse import bacc

    F32 = mybir.dt.float32
    BF16 = mybir.dt.bfloat16

    nc = bacc.Bacc("TRN2", target_bir_lowering=False, debug=False,
                   num_devices=N_CORES)

    qk_dt = BF16 if QK_BF16 else F32
    q_d = nc.dram_tensor("q", [HPC, D, S], qk_dt, kind="ExternalInput").ap()
    k_d = nc.dram_tensor("k", [HPC, D, S], qk_dt, kind="ExternalInput").ap()
    vt_d = nc.dram_tensor("vt", [HPC, D, S], BF16, kind="ExternalInput").ap()
    u_d = nc.dram_tensor("u", [NSTEP, D, JHALF], BF16,
                         kind="ExternalOutput").ap()
    c_d = nc.dram_tensor("c", [NSTEP, 128, 8], F32,
                         kind="ExternalOutput").ap()

    with tile.TileContext(nc) as tc:
        with ExitStack() as ctx:
            cpool = ctx.enter_context(tc.tile_pool(name="const", bufs=1))
            qpool = ctx.enter_context(tc.tile_pool(name="q", bufs=5))
            kpool = ctx.enter_context(tc.tile_pool(name="k", bufs=5))
            vtpool = ctx.enter_context(tc.tile_pool(name="vt", bufs=5))
            epool = ctx.enter_context(tc.tile_pool(name="e", bufs=10))
            esumpool = ctx.enter_context(tc.tile_pool(name="esum", bufs=2))
            ctpool = ctx.enter_context(tc.tile_pool(name="ct", bufs=2))
            usbpool = ctx.enter_context(tc.tile_pool(name="usb", bufs=2))
            spool = ctx.enter_context(
                tc.tile_pool(name="spsum", bufs=2, space="PSUM"))
            upool = ctx.enter_context(
                tc.tile_pool(name="upsum", bufs=1, space="PSUM"))

            first = _make_prefetch(nc, locals())(0, 0)

            ones_col = cpool.tile([128, 1], mybir.dt.bfloat16, tag="ones_col")
            nc.vector.memset(ones_col[:], 1.0)

            loop_cm = (tc.For_i(0, loop_n, 1) if loop_n
                       else contextlib.nullcontext())
            with loop_cm:
                _emit_body(nc, tc, locals(), first=first)

    nc.compile()
    return nc


def _make_prefetch(nc, env):
    import concourse.mybir as mybir
    F32R = mybir.dt.float32r
    BF16 = mybir.dt.bfloat16
    qpool, kpool, vtpool = env["qpool"], env["kpool"], env["vtpool"]
    q_d, k_d, vt_d = env["q_d"], env["k_d"], env["vt_d"]

    QKDT = BF16 if QK_BF16 else F32R

    def prefetch(h, jh):
        # split loads across DMA queues so early chunks' inputs land before
        # the whole transfer completes
        k = kpool.tile([128, JHALF], QKDT, tag="k")
        for p in range(2):
            nc.sync.dma_start(
                k[:, p * 512:(p + 1) * 512],
                k_d[h, :, jh * JHALF + p * 512:
                    jh * JHALF + (p + 1) * 512].bitcast(QKDT))
        q = qpool.tile([128, S], QKDT, tag="q")
        for p in range(4):
            nc.sync.dma_start(
                q[:, p * 512:(p + 1) * 512],
                q_d[h, :, p * 512:(p + 1) * 512].bitcast(QKDT))
        vt = vtpool.tile([128, S], BF16, tag="vt")
        for p in range(2):
            nc.sync.dma_start(
                vt[:, p * 1024:(p + 1) * 1024],
                vt_d[h, :, p * 1024:(p + 1) * 1024])
        return q, k, vt

    return prefetch


def _emit_body(nc, tc, env, first=None):
    """Software-pipelined emission over steps (h, jh), piece stream
    g = 2*ic + js inside each step, FUSE pieces per PSUM tile / Exp op.

    Per-step tail (colsumT matmuls + U/colsum DMA-out) is woven into the
    next step's big-tile stream."""
    import concourse.mybir as mybir

    F32 = mybir.dt.float32
    BF16 = mybir.dt.bfloat16
    EXP = mybir.ActivationFunctionType.Exp
    qpool, kpool, vtpool = env["qpool"], env["kpool"], env["vtpool"]
    epool, esumpool = env["epool"], env["esumpool"]
    ctpool, usbpool = env["ctpool"], env["usbpool"]
    spool, upool = env["spool"], env["upool"]
    ones_col = env["ones_col"]
    u_d, c_d = env["u_d"], env["c_d"]

    prefetch = _make_prefetch(nc, env)

    class Head:
        pass

    def tail_piece(st, piece):
        op = TAIL_OPS[piece]
        if op == "colsum":
            # short borrow of a score-ring slot: 8 tiny matmuls produce
            # colsumT[p, jb] = sum_i esum[i, jb*128+p]
            st.ct = spool.tile([128, FUSE * 512], F32, tag="s")
            for jb in range(8):
                nc.tensor.matmul(
                    st.ct[:, jb:jb + 1],
                    lhsT=st.esum[:, jb * 128:(jb + 1) * 128],
                    rhs=ones_col[:],
                    start=True, stop=True)
        elif op == "ct_sb":
            st.ct_sb = ctpool.tile([128, 8], F32, tag="ct_sb")
            nc.vector.tensor_copy(st.ct_sb[:], st.ct[:, 0:8])
        elif op == "dma_c":
            nc.sync.dma_start(c_d[st.si], st.ct_sb[:])
        elif op == "u_sb":
            st.u_sb = usbpool.tile([128, JHALF], BF16, tag="u_sb")
            nc.vector.tensor_copy(st.u_sb[:], st.u[:])
        elif op == "dma_u":
            nc.sync.dma_start(u_d[st.si], st.u_sb[:])

    TAIL_OPS = ["colsum", "ct_sb", "dma_c", "u_sb", "dma_u"]
    NTAIL = len(TAIL_OPS)
    TAIL_BIG = TAIL_SCHED
    steps = [(h, jh) for jh in range(JH) for h in range(HPC)]
    cur = first if first is not None else prefetch(*steps[0])
    nxt = None
    pend = None
    tail_next = NTAIL
    mm2q = []  # lagged (vt, e_tile, e_off, u, g, wide) mm2 work

    def emit_mm2(ent):
        vt, e_tile, e_off, u, g, wide = ent
        ic, js = g // 2, g % 2
        w = 1024 if wide else 512
        nc.tensor.matmul(
            u[:, js * 512:js * 512 + w],
            lhsT=vt[:, ic * 128:(ic + 1) * 128],
            rhs=e_tile[:, e_off:e_off + w],
            start=(ic == 0), stop=(ic == IC - 1))

    for si, (h, jh) in enumerate(steps):
        q, k, vt = cur
        u = upool.tile([128, JHALF], F32, tag="u")
        esum = esumpool.tile([128, JHALF], BF16, tag="esum")

        for b in range(NBIG):
            npc = min(FUSE, NPIECE - b * FUSE)
            sb = spool.tile([128, FUSE * 512], F32, tag="s")
            for p in range(npc):
                gg = b * FUSE + p
                ic, js = gg // 2, gg % 2
                nc.tensor.matmul(
                    sb[:, p * 512:(p + 1) * 512],
                    lhsT=q[:, ic * 128:(ic + 1) * 128],
                    rhs=k[:, js * 512:(js + 1) * 512],
                    start=True, stop=True)
            eb = epool.tile([128, FUSE * 512], BF16, tag="e")
            nc.scalar.activation(eb[:, 0:npc * 512], sb[:, 0:npc * 512],
                                 EXP, scale=SCALE)
            for p in range(npc):
                gg = b * FUSE + p
                mm2q.append((vt, eb, p * 512, u, gg, False))
            # runway: during the first RUNWAY bigs of a step, let the mm2
            # queue grow so the PE stream leads with mm1 work and the
            # activation engine never starves across the step boundary
            if b >= RUNWAY:
                while len(mm2q) > LAG:
                    emit_mm2(mm2q.pop(0))
            # previous step's tail
            while (pend is not None and tail_next < NTAIL
                   and b >= TAIL_BIG[tail_next]):
                tail_piece(pend, tail_next)
                tail_next += 1
            # esum accumulation: coalesce adjacent pieces that map to a
            # contiguous [0,1024) esum range (js pairs)
            p = 0
            while p < npc:
                gg = b * FUSE + p
                js = gg % 2
                if js == 0 and p + 1 < npc:
                    dst = esum[:, 0:1024]
                    src = eb[:, p * 512:(p + 2) * 512]
                    wide = True
                else:
                    dst = esum[:, js * 512:(js + 1) * 512]
                    src = eb[:, p * 512:(p + 1) * 512]
                    wide = False
                if gg <= 1:
                    nc.vector.tensor_copy(dst, src)
                else:
                    nc.vector.tensor_add(dst, dst, src)
                p += 2 if wide else 1
            if b == 1 and si + 1 < len(steps):
                nxt = prefetch(*steps[si + 1])
        assert pend is None or tail_next >= NTAIL
        st = Head()
        st.esum, st.u, st.si = esum, u, si
        pend, tail_next = st, 0
        cur = nxt

    # drain: remaining lagged mm2s, then the last step's tail
    for ent in mm2q:
        emit_mm2(ent)
    for piece in range(NTAIL):
        tail_piece(pend, piece)


def _get_nc():
    if "nc" not in _COMPILED:
        _COMPILED["nc"] = _build_nc()
    return _COMPILED["nc"]


def _prep_inputs(Q, K, V, W=None):
    """Slice + lay out per-core inputs on host."""
    bf16 = ml_dtypes.bfloat16
    Q = np.ascontiguousarray(Q, dtype=np.float32)
    K = np.ascontiguousarray(K, dtype=np.float32)
    V = np.ascontiguousarray(V, dtype=np.float32)

    in_maps = []
    for c in range(N_CORES):
        qs = Q[c * HPC:(c + 1) * HPC]
        ks = K[c * HPC:(c + 1) * HPC]
        vs = V[c * HPC:(c + 1) * HPC]
        # vt[h][i_sub, ic*128 + d] = V[h, d, ic*128 + i_sub]
        vt = (vs.reshape(HPC, D, IC, 128)
              .transpose(0, 3, 2, 1)
              .reshape(HPC, 128, S)
              .astype(bf16))
        if QK_BF16:
            qs = qs.astype(bf16)
            ks = ks.astype(bf16)
        in_maps.append({
            "q": np.ascontiguousarray(qs),
            "k": np.ascontiguousarray(ks),
            "vt": np.ascontiguousarray(vt),
        })
    return in_maps


def _run(in_maps, trace=False):
    from concourse.bass_utils import run_bass_kernel_spmd
    nc = _get_nc()
    return run_bass_kernel_spmd(nc, in_maps, list(range(N_CORES)), trace=trace)


def kernel(x, Q, K, V, W, _trace=False, _return_result=False):
    in_maps = _prep_inputs(Q, K, V)
    res = _run(in_maps, trace=_trace)

    W = np.ascontiguousarray(W, dtype=np.float32)
    Wr = W.reshape(N_HEADS, D, D)  # [h, d, k]
    out = np.zeros((B, D, S), dtype=np.float32)
    for c in range(N_CORES):
        uc = res.results[c]["u"].astype(np.float32)
        cc = res.results[c]["c"]  # [NSTEP, 128, 8] f32
        b = c // 2
        us = np.empty((HPC, D, S), dtype=np.float32)
        for si in range(NSTEP):
            h, jh = si % HPC, si // HPC
            # csum[j] lives at colsumT[j % 128, j // 128]
            r = 1.0 / cc[si].T.reshape(JHALF)
            us[h, :, jh * JHALF:(jh + 1) * JHALF] = uc[si] * r
        heads = Wr[(c % 2) * HPC:(c % 2 + 1) * HPC]  # [HPC, d, k]
        out[b] += np.einsum("hdk,hds->ks", heads, us, optimize=True)
    if _return_result:
        return out, res
    return out


# revision 26
# speedup vs baseline: 1.1221x; 1.1221x over previous
"""Trainium2 Bass kernel for multi-head enc-dec attention with softmax over
the query axis (legacy F.softmax(dim=1) on [N, S, S]) plus an output
projection.

Math (per head n):
    S[i, j]  = sum_d Q[n, d, i] * K[n, d, j] / sqrt(128)
    E        = exp(S)                      (softmax over i == axis 0)
    U[d, j]  = sum_i V[n, d, i] * E[i, j]
    out_h    = U / colsum(E)               (colsum over i, per j)
    result[b] = sum_{heads h of b} W_h^T @ out_h

Sharding: N = 64 heads split across 8 cores (8 heads each). No collectives.

v4 design (device = the 99.2% of FLOPs that are S-sized; host = the
final per-j normalization + d_model projection, 0.8% of FLOPs):
  - The scalar engine (exp of all 4.2M scores per head) is the hard
    bottleneck at ~456ns/512 cols + 316ns/instruction (measured), so the
    schedule exists to keep ScalarE saturated: scores are computed in
    [128, 512] pieces packed 3-per-[128,1536]-PSUM-tile so each Exp
    instruction covers 1536 columns.
  - mm2 (V^T E) accumulates per-head U in PSUM; adjacent (js0, js1)
    pieces of one i-chunk are paired into single 1024-wide matmuls.
  - colsum of E via 8 tiny ones-matmuls (lhsT = esum 128-slices) into a
    [128, 8] PSUM tile => per-j sums land transposed; DMA'd out raw.
  - U and colsumT stream to DRAM; the host does r = 1/colsum,
    out_b += W_h^T (U_h * r) in fp32 (also slightly more accurate than
    a device-side bf16 projection).
PSUM budget: 2 x 3 banks scores + 2 banks U = 8; the colsumT tile
briefly borrows a score-ring slot (lifetime < 2 exp ops).
"""

import math
from contextlib import ExitStack

import ml_dtypes
import numpy as np

N_CORES = 8
B, N_HEADS, D, S = 4, 16, 128, 2048
HPC = (B * N_HEADS) // N_CORES  # heads per core = 8
IC = S // 128  # 16 i-chunks
JH = 2  # j halves
JHALF = S // JH  # 1024
NSTEP = HPC * JH
NPIECE = 2 * IC  # 512-wide (ic, js) score pieces per step
FUSE = 3  # pieces per PSUM score tile / Exp op
NBIG = (NPIECE + FUSE - 1) // FUSE
SCALE = 1.0 / math.sqrt(D)
LAG = 12  # mm2 piece lag
RUNWAY = 3  # bigs at step start during which mm2 drain is paused
TAIL_SCHED = [1, 2, 3, 4]  # big (in next step) per tail piece
QK_BF16 = True

_COMPILED = {}


def _build_nc(loop_n=None):
    """loop_n: if set, wrap the body in a device-side For_i that repeats it
    loop_n times (used only for wall-clock-difference HW timing)."""
    import contextlib
    import concourse.mybir as mybir
    import concourse.tile as tile
    from concourse import bacc

    F32 = mybir.dt.float32
    BF16 = mybir.dt.bfloat16

    nc = bacc.Bacc("TRN2", target_bir_lowering=False, debug=False,
                   num_devices=N_CORES)

    qk_dt = BF16 if QK_BF16 else F32
    q_d = nc.dram_tensor("q", [HPC, D, S], qk_dt, kind="ExternalInput").ap()
    k_d = nc.dram_tensor("k", [HPC, D, S], qk_dt, kind="ExternalInput").ap()
    vt_d = nc.dram_tensor("vt", [HPC, D, S], BF16, kind="ExternalInput").ap()
    u_d = nc.dram_tensor("u", [HPC, D, S], BF16,
                         kind="ExternalOutput").ap()
    c_d = nc.dram_tensor("c", [HPC, 128, 2 * 8], F32,
                         kind="ExternalOutput").ap()

    with tile.TileContext(nc) as tc:
        with ExitStack() as ctx:
            cpool = ctx.enter_context(tc.tile_pool(name="const", bufs=1))
            qpool = ctx.enter_context(tc.tile_pool(name="q", bufs=5))
            kpool = ctx.enter_context(tc.tile_pool(name="k", bufs=5))
            vtpool = ctx.enter_context(tc.tile_pool(name="vt", bufs=5))
            epool = ctx.enter_context(tc.tile_pool(name="e", bufs=10))
            esumpool = ctx.enter_context(tc.tile_pool(name="esum", bufs=2))
            ctpool = ctx.enter_context(tc.tile_pool(name="ct", bufs=2))
            usbpool = ctx.enter_context(tc.tile_pool(name="usb", bufs=2))
            spool = ctx.enter_context(
                tc.tile_pool(name="spsum", bufs=2, space="PSUM"))
            upool = ctx.enter_context(
                tc.tile_pool(name="upsum", bufs=1, space="PSUM"))

            first = _make_prefetch(nc, locals())(0)

            ones_col = cpool.tile([128, 1], mybir.dt.bfloat16, tag="ones_col")
            nc.vector.memset(ones_col[:], 1.0)

            loop_cm = (tc.For_i(0, loop_n, 1) if loop_n
                       else contextlib.nullcontext())
            with loop_cm:
                _emit_body(nc, tc, locals(), first=first)

    nc.compile()
    return nc


def _make_prefetch(nc, env):
    import concourse.mybir as mybir
    F32R = mybir.dt.float32r
    BF16 = mybir.dt.bfloat16
    qpool, kpool, vtpool = env["qpool"], env["kpool"], env["vtpool"]
    q_d, k_d, vt_d = env["q_d"], env["k_d"], env["vt_d"]

    QKDT = BF16 if QK_BF16 else F32R
    import os
    noio = bool(os.environ.get("BENCH_NOIO"))

    def prefetch(h):
        k = kpool.tile([128, S], QKDT, tag="k")
        q = qpool.tile([128, S], QKDT, tag="q")
        vt = vtpool.tile([128, S], BF16, tag="vt")
        if noio:
            # timing experiment only: load a sliver so tiles have data deps
            nc.sync.dma_start(k[:, 0:64], k_d[h, :, 0:64].bitcast(QKDT))
            nc.sync.dma_start(q[:, 0:64], q_d[h, :, 0:64].bitcast(QKDT))
            nc.sync.dma_start(vt[:, 0:64], vt_d[h, :, 0:64])
            return q, k, vt
        nc.sync.dma_start(k[:], k_d[h].bitcast(QKDT))
        nc.sync.dma_start(q[:], q_d[h].bitcast(QKDT))
        nc.sync.dma_start(vt[:], vt_d[h])
        return q, k, vt

    return prefetch


def _emit_body(nc, tc, env, first=None):
    """Software-pipelined emission over steps (h, jh), piece stream
    g = 2*ic + js inside each step, FUSE pieces per PSUM tile / Exp op.

    Per-step tail (colsumT matmuls + U/colsum staging) is woven into the
    next step's big-tile stream; DMA-out once per head on the Pool DGE
    queue (keeps the SP queue free for input prefetch)."""
    import os
    import concourse.mybir as mybir

    F32 = mybir.dt.float32
    BF16 = mybir.dt.bfloat16
    EXP = mybir.ActivationFunctionType.Exp
    qpool, kpool, vtpool = env["qpool"], env["kpool"], env["vtpool"]
    epool, esumpool = env["epool"], env["esumpool"]
    ctpool, usbpool = env["ctpool"], env["usbpool"]
    spool, upool = env["spool"], env["upool"]
    ones_col = env["ones_col"]
    u_d, c_d = env["u_d"], env["c_d"]
    noio = bool(os.environ.get("BENCH_NOIO"))

    prefetch = _make_prefetch(nc, env)

    class Head:
        pass

    def tail_piece(st, piece):
        op = TAIL_OPS[piece]
        if op == "colsum":
            # short borrow of a score-ring slot: 8 tiny matmuls produce
            # colsumT[p, jb] = sum_i esum[i, jb*128+p]
            st.ct = spool.tile([128, FUSE * 512], F32, tag="s")
            for jb in range(8):
                nc.tensor.matmul(
                    st.ct[:, jb:jb + 1],
                    lhsT=st.esum[:, jb * 128:(jb + 1) * 128],
                    rhs=ones_col[:],
                    start=True, stop=True)
        elif op == "ct_sb":
            if st.jh == 0:
                st.head.ct_sb = ctpool.tile([128, 2 * 8], F32, tag="ct_sb", name="ct_sb")
            nc.vector.tensor_copy(
                st.head.ct_sb[:, st.jh * 8:(st.jh + 1) * 8], st.ct[:, 0:8])
        elif op == "u_sb":
            if st.jh == 0:
                st.head.u_sb = usbpool.tile([128, S], BF16, tag="u_sb", name="u_sb")
            nc.vector.tensor_copy(
                st.head.u_sb[:, st.jh * JHALF:(st.jh + 1) * JHALF], st.u[:])
        elif op == "dma_out":
            if st.jh == 1:
                if noio:
                    nc.gpsimd.dma_start(u_d[st.h][:, 0:64],
                                        st.head.u_sb[:, 0:64])
                else:
                    nc.gpsimd.dma_start(u_d[st.h], st.head.u_sb[:])
                nc.gpsimd.dma_start(c_d[st.h], st.head.ct_sb[:])

    TAIL_OPS = ["colsum", "ct_sb", "u_sb", "dma_out"]
    NTAIL = len(TAIL_OPS)
    TAIL_BIG = TAIL_SCHED
    steps = [(h, jh) for h in range(HPC) for jh in range(JH)]
    cur = first if first is not None else prefetch(steps[0][0])
    nxt = None
    pend = None
    tail_next = NTAIL
    mm2q = []  # lagged (vt, e_tile, e_off, u, g) mm2 work
    head = None

    def emit_mm2(ent):
        vt, e_tile, e_off, u, g = ent
        ic, js = g // 2, g % 2
        nc.tensor.matmul(
            u[:, js * 512:(js + 1) * 512],
            lhsT=vt[:, ic * 128:(ic + 1) * 128],
            rhs=e_tile[:, e_off:e_off + 512],
            start=(ic == 0), stop=(ic == IC - 1))

    for si, (h, jh) in enumerate(steps):
        if jh == 0:
            head = Head()
        q, k, vt = cur
        u = upool.tile([128, JHALF], F32, tag="u")
        esum = esumpool.tile([128, JHALF], BF16, tag="esum")

        for b in range(NBIG):
            npc = min(FUSE, NPIECE - b * FUSE)
            sb = spool.tile([128, FUSE * 512], F32, tag="s")
            for p in range(npc):
                gg = b * FUSE + p
                ic, js = gg // 2, gg % 2
                nc.tensor.matmul(
                    sb[:, p * 512:(p + 1) * 512],
                    lhsT=q[:, ic * 128:(ic + 1) * 128],
                    rhs=k[:, jh * JHALF + js * 512:jh * JHALF + (js + 1) * 512],
                    start=True, stop=True)
            eb = epool.tile([128, FUSE * 512], BF16, tag="e")
            nc.scalar.activation(eb[:, 0:npc * 512], sb[:, 0:npc * 512],
                                 EXP, scale=SCALE)
            for p in range(npc):
                gg = b * FUSE + p
                mm2q.append((vt, eb, p * 512, u, gg))
            # runway: during the first RUNWAY bigs of a step, let the mm2
            # queue grow so the PE stream leads with mm1 work and the
            # activation engine never starves across the step boundary
            if b >= RUNWAY:
                while len(mm2q) > LAG:
                    emit_mm2(mm2q.pop(0))
            # previous step's tail
            while (pend is not None and tail_next < NTAIL
                   and b >= TAIL_BIG[tail_next]):
                tail_piece(pend, tail_next)
                tail_next += 1
            # esum accumulation: coalesce adjacent pieces that map to a
            # contiguous [0,1024) esum range (js pairs)
            p = 0
            while p < npc:
                gg = b * FUSE + p
                js = gg % 2
                if js == 0 and p + 1 < npc:
                    dst = esum[:, 0:1024]
                    srcv = eb[:, p * 512:(p + 2) * 512]
                    wide = True
                else:
                    dst = esum[:, js * 512:(js + 1) * 512]
                    srcv = eb[:, p * 512:(p + 1) * 512]
                    wide = False
                if gg <= 1:
                    nc.vector.tensor_copy(dst, srcv)
                else:
                    nc.vector.tensor_add(dst, dst, srcv)
                p += 2 if wide else 1
            if b == 1 and jh == 0 and h + 1 < HPC:
                nxt = prefetch(h + 1)
        assert pend is None or tail_next >= NTAIL
        st = Head()
        st.esum, st.u, st.h, st.jh, st.head = esum, u, h, jh, head
        pend, tail_next = st, 0
        if jh == 1:
            cur = nxt

    # drain: remaining lagged mm2s, then the last step's tail
    for ent in mm2q:
        emit_mm2(ent)
    for piece in range(NTAIL):
        tail_piece(pend, piece)


def _get_nc():
    if "nc" not in _COMPILED:
        _COMPILED["nc"] = _build_nc()
    return _COMPILED["nc"]


def _prep_inputs(Q, K, V, W=None):
    """Slice + lay out per-core inputs on host."""
    bf16 = ml_dtypes.bfloat16
    Q = np.ascontiguousarray(Q, dtype=np.float32)
    K = np.ascontiguousarray(K, dtype=np.float32)
    V = np.ascontiguousarray(V, dtype=np.float32)

    in_maps = []
    for c in range(N_CORES):
        qs = Q[c * HPC:(c + 1) * HPC]
        ks = K[c * HPC:(c + 1) * HPC]
        vs = V[c * HPC:(c + 1) * HPC]
        # vt[h][i_sub, ic*128 + d] = V[h, d, ic*128 + i_sub]
        vt = (vs.reshape(HPC, D, IC, 128)
              .transpose(0, 3, 2, 1)
              .reshape(HPC, 128, S)
              .astype(bf16))
        if QK_BF16:
            qs = qs.astype(bf16)
            ks = ks.astype(bf16)
        in_maps.append({
            "q": np.ascontiguousarray(qs),
            "k": np.ascontiguousarray(ks),
            "vt": np.ascontiguousarray(vt),
        })
    return in_maps


def _run(in_maps, trace=False):
    from concourse.bass_utils import run_bass_kernel_spmd
    nc = _get_nc()
    return run_bass_kernel_spmd(nc, in_maps, list(range(N_CORES)), trace=trace)


def kernel(x, Q, K, V, W, _trace=False, _return_result=False):
    in_maps = _prep_inputs(Q, K, V)
    res = _run(in_maps, trace=_trace)

    W = np.ascontiguousarray(W, dtype=np.float32)
    Wr = W.reshape(N_HEADS, D, D)  # [h, d, k]
    out = np.zeros((B, D, S), dtype=np.float32)
    for c in range(N_CORES):
        uc = res.results[c]["u"].astype(np.float32)  # [HPC, 128, S]
        cc = res.results[c]["c"]  # [HPC, 128, 16]
        b = c // 2
        us = np.empty((HPC, D, S), dtype=np.float32)
        for h in range(HPC):
            for jh in range(JH):
                # csum[j] lives at colsumT[j % 128, jh*8 + j // 128]
                r = 1.0 / cc[h][:, jh * 8:(jh + 1) * 8].T.reshape(JHALF)
                us[h, :, jh * JHALF:(jh + 1) * JHALF] = \
                    uc[h][:, jh * JHALF:(jh + 1) * JHALF] * r
        heads = Wr[(c % 2) * HPC:(c % 2 + 1) * HPC]  # [HPC, d, k]
        out[b] += np.einsum("hdk,hds->ks", heads, us, optimize=True)
    if _return_result:
        return out, res
    return out
